# revision 3
# baseline (speedup 1.0000x reference)
"""AtomGMMProjector Bass kernel for Trainium2 (8 NeuronCores, SPMD).
v3: 2D-bucketed windows, PE-built z, 5-engine balance.

Math (per batch b):
    out[y, x] = sum_n amp[n] * exp(-zy(n,y)^2) * exp(-zx(n,x)^2)
    z*(n, d)  = s[n]*line[d] + b*[n],  s = 1/(sqrt2*sigma), b* = -c*_n*s

Structure (per 128-atom tile, windows precomputed on host):
  * zy on PE: one K=4 bf16 matmul  {s_hi,s_lo,by_hi,by_lo}^T @ {line,line,1,1}
    -> PSUM f32.  The hi/lo split keeps z exact to ~2^-17 while running at
    the 1-cycle/row bf16 rate (f32 matmuls are 4x slower).
  * zx on DVE: tensor_scalar line*s+bx (bf16 4x mode, f32 per-partition
    scalars).
  * erf' (Derivative_Erf = 2/sqrt(pi) exp(-x^2)): ONE batched ACT op per
    axis per batch (PSUM-source for y, SBUF for x) -> bf16.
  * ax = ex*amp' on DVE/Pool (split tuned); amp' = amp*pi/4 cancels the
    (2/sqrt(pi))^2 from the two erf' factors.
  * Accumulating bf16 matmuls into ONE [128, 512] f32 PSUM bank per batch
    (cols 0:256 = y in [0,128), 256:512 = y in [128,256)).  PE writes land
    on 32-partition quadrant boundaries only, so y-windows have 32-aligned
    low edges and each tile's matmul is emitted per 32-slab.
  * PSUM -> SBUF out-copy split DVE/Pool, single HBM DMA per batch.

Atoms are 2D-bucketed per batch on host (adaptive k-d: median split on the
wider axis), so both axes' windows are narrow.  All coefficients
(s, bx, by, amp') are host-precomputed and DMA'd as contiguous
partition-major blocks.  Windows are unioned over each body-slot's 8
possible batches (core c runs batches 4c..4c+3; slot j serves batches
{j, j+4, ...}), so one SPMD program works for all cores.
"""

import numpy as np
from contextlib import ExitStack

import concourse.bass as bass
import concourse.bacc as bacc
import concourse.mybir as mybir
import concourse.tile as tile
from concourse.bass_utils import run_bass_kernel_spmd

F32 = mybir.dt.float32
BF16 = mybir.dt.bfloat16
AF = mybir.ActivationFunctionType
OP = mybir.AluOpType

import os
B, N, D = 32, 4096, 256
NCORES = 8
BPC = B // NCORES
P = 128
NT = N // P
MARGIN_SIG = float(os.environ.get("K_MARGIN", "3.5"))  # window margin, units of per-tile max sigma
QX = 4                       # x window quantization
QYH = 4                      # y high-edge quantization
QYL = 32                     # y low-edge: PE quadrant alignment
PI_4 = 0.7853981633974483

AX_DVE = int(os.environ.get("K_AX_DVE", "8"))    # per 16 tiles, ax ops on DVE
NGRP = int(os.environ.get("K_NGRP", "2"))        # erf groups per axis per batch
ZY_BUFS = int(os.environ.get("K_ZY_BUFS", "2"))
ZX_DVE = int(os.environ.get("K_ZX_DVE", "16"))   # per 16 tiles, zx ops on DVE (rest Pool)
YPEN = float(os.environ.get("K_YPEN", "1.15"))
PS_BUFS = int(os.environ.get("K_PS_BUFS", "2"))


def _bf(a):
    return a.astype(mybir.dt.np(BF16))


# ---------------------------------------------------------------------------
# Host prep
# ---------------------------------------------------------------------------

def _kd_order(cx, cy, levels=5, ypen=YPEN):
    """Adaptive k-d bucketing: at each node, split at the median of the axis
    whose split minimizes the projected window cost (clamped extents +
    margins, unioned across batches).  One split tree shared by all batches
    (the SPMD program is shared), leaf membership per batch."""
    GRID = 128.0  # grid half-width: windows clamp to [-128, 128)

    def ext(c_ax, nodes):
        # unioned, grid-clamped extent of a node along one axis
        lo = np.mean([np.clip(c_ax[b][i], -GRID, GRID).min()
                      for b, i in enumerate(nodes)])
        hi = np.mean([np.clip(c_ax[b][i], -GRID, GRID).max()
                      for b, i in enumerate(nodes)])
        return hi - lo

    def trysplit(c_ax, nodes):
        lo, hi = [], []
        for b, i in enumerate(nodes):
            srt = i[np.argsort(c_ax[b][i], kind="stable")]
            h = len(srt) // 2
            lo.append(srt[:h])
            hi.append(srt[h:])
        return lo, hi

    def cost(children):
        # per-child projected leaf cost: after further splits a child's
        # extent shrinks along whichever axis gets split, so score by the
        # axis sum (what the final windows integrate over)
        t = 0.0
        for ch in children:
            t += ext(cx, [ch[b] for b in range(B)]) if False else 0
        return t

    def split(nodes, depth):
        if depth == 0:
            return [nodes]
        sx = trysplit(cx, nodes)
        sy = trysplit(cy, nodes)
        # projected cost: sum over both children of (ext_x + ext_y), with the
        # y axis weighted for its 32-alignment penalty
        cxx = sum(ext(cx, s) + ypen * ext(cy, s) for s in sx)
        cyy = sum(ext(cx, s) + ypen * ext(cy, s) for s in sy)
        pick = sx if cxx <= cyy else sy
        return split(pick[0], depth - 1) + split(pick[1], depth - 1)

    leaves = split([np.arange(N) for _ in range(B)], levels)
    order = np.empty((B, N), np.int64)
    for b in range(B):
        order[b] = np.concatenate([leaf[b] for leaf in leaves])
    return order


def prep(line_coords, rot_mats, centers, sigmas, amplitudes):
    line = np.asarray(line_coords, np.float64)
    base, step = line[0], line[1] - line[0]
    cx = np.einsum("bj,bnj->bn", rot_mats[:, 0, :], centers).astype(np.float64)
    cy = np.einsum("bj,bnj->bn", rot_mats[:, 1, :], centers).astype(np.float64)
    order = _kd_order(cx, cy)

    sx = np.take_along_axis(cx, order, 1)
    sy = np.take_along_axis(cy, order, 1)
    ss = np.asarray(sigmas, np.float64)[order]
    sa = np.asarray(amplitudes, np.float64)[order]

    # per body-slot windows: slot j is run for batches {j, j+BPC, ...}
    def win(c, slot_batches, q0, q1):
        wins = []
        for t in range(NT):
            seg = slice(P * t, P * (t + 1))
            m = MARGIN_SIG * ss[slot_batches, seg].max(axis=1)
            lo = ((c[slot_batches, seg].min(axis=1) - m - base) / step).min()
            hi = ((c[slot_batches, seg].max(axis=1) + m - base) / step).max()
            w0 = max(0, min(D, int(np.floor(lo / q0) * q0)))
            w1 = max(0, min(D, int(np.ceil((hi + 1.0) / q1) * q1)))
            wins.append((w0, w1))
        return wins

    xwins, ywins = [], []
    for j in range(BPC):
        sb = list(range(j, B, BPC))
        xwins.append(win(sx, sb, QX, QX))
        ywins.append(win(sy, sb, QYL, QYH))

    s = 1.0 / (np.sqrt(2.0) * ss)
    bx = -sx * s
    by = -sy * s
    ap = sa * PI_4

    # coef [B, 128, 3*NT] f32: blocks [s | bx | amp'], atom (t, p) = 128t+p
    coef = np.empty((B, P, 3 * NT), np.float32)
    for k, v in enumerate((s, bx, ap)):
        coef[:, :, k * NT:(k + 1) * NT] = v.reshape(B, NT, P).transpose(0, 2, 1)

    # sbp4 [B, 4, N] bf16: rows {s_hi, s_lo, by_hi, by_lo} (PE zy lhsT)
    s_hi = _bf(s).astype(np.float64)
    by_hi = _bf(by).astype(np.float64)
    sbp4 = np.stack([s_hi, s - s_hi, by_hi, by - by_hi], axis=1)
    sbp4 = _bf(sbp4.astype(np.float32))                     # (B, 4, N)

    lineBF = _bf(np.tile(np.asarray(line_coords, np.float32)[None, :], (P, 1)))
    l32 = np.asarray(line_coords, np.float32)
    line4 = _bf(np.stack([l32, l32, np.ones(D, np.float32),
                          np.ones(D, np.float32)], axis=0))  # (4, D)
    return order, xwins, ywins, coef, sbp4, lineBF, line4


# ---------------------------------------------------------------------------
# Device program
# ---------------------------------------------------------------------------

def build(wins, repeats=1):
    xwins, ywins = wins
    nc = bacc.Bacc("TRN2", target_bir_lowering=False, debug=False)
    lineBF_d = nc.dram_tensor("lineBF", [P, D], BF16, kind="ExternalInput")
    coef_d = nc.dram_tensor("coef", [P, BPC * 3 * NT], F32, kind="ExternalInput")
    sbp4_d = nc.dram_tensor("sbp4", [4, BPC * N], BF16, kind="ExternalInput")
    out_d = nc.dram_tensor("out", [BPC, D, D], F32, kind="ExternalOutput")

    with tile.TileContext(nc) as tc, ExitStack() as ctx:
        nc_ = tc.nc
        constp = ctx.enter_context(tc.tile_pool(name="const", bufs=1))
        coefp = ctx.enter_context(tc.tile_pool(name="coef", bufs=BPC))
        zxp = ctx.enter_context(tc.tile_pool(name="zx", bufs=3))
        exp_ = ctx.enter_context(tc.tile_pool(name="ex", bufs=3))
        eyp = ctx.enter_context(tc.tile_pool(name="ey", bufs=3))
        axp = ctx.enter_context(tc.tile_pool(name="ax", bufs=12))
        osbp = ctx.enter_context(tc.tile_pool(name="osb", bufs=2))
        zyp = ctx.enter_context(tc.tile_pool(name="zy", bufs=ZY_BUFS, space="PSUM"))
        psp = ctx.enter_context(tc.tile_pool(name="ps", bufs=PS_BUFS, space="PSUM"))

        lineBF_t = constp.tile([P, D], BF16)
        nc_.sync.dma_start(lineBF_t[:], lineBF_d.ap())
        line4_t = constp.tile([4, D], BF16)
        nc_.vector.memset(line4_t[:], 1.0)
        nc_.vector.tensor_copy(line4_t[0:2, :], lineBF_t[0:2, :])
        zrow_t = constp.tile([1, 2 * D], BF16)
        nc_.vector.memset(zrow_t[:], 0.0)

        for _ in range(repeats):
            _body(tc, coefp, zxp, exp_, eyp, axp, osbp, zyp, psp,
                  lineBF_t, line4_t, zrow_t, coef_d.ap(), sbp4_d.ap(),
                  out_d.ap(), xwins, ywins)
    nc.compile()
    return nc


def _body(tc, coefp, zxp, exp_, eyp, axp, osbp, zyp, psp,
          lineBF_t, line4_t, zrow_t, coef, sbp4, out, xwins, ywins):
    nc = tc.nc

    # one DMA each for all batches' coefficients (contiguous host layouts)
    sbp_all = coefp.tile([4, BPC * N], BF16, tag="sbp")
    nc.sync.dma_start(sbp_all[:], sbp4)
    coef_all = coefp.tile([P, BPC * 3 * NT], F32, tag="coef")
    nc.sync.dma_start(coef_all[:], coef)
    
    per_batch = [(coef_all[:, b * 3 * NT:(b + 1) * 3 * NT],
                  sbp_all[:, b * N:(b + 1) * N])
                 for b in range(BPC)]

    stages = [_mk_stages(tc, b, coefp, zxp, exp_, eyp, axp, osbp, zyp, psp,
                         lineBF_t, line4_t, zrow_t, per_batch[b], out,
                         xwins[b], ywins[b]) for b in range(BPC)]
    # software pipeline over (batch, group) units, depth 2: each engine's
    # in-order stream has unit u's producers (PE zy / DVE zx) ahead of unit
    # u-1's erf (ACT) ahead of unit u-2's consumers (ax+slab matmuls), so
    # ACT never waits on a producer and PE never runs dry
    units = [(b, g) for b in range(BPC)
             for g in range(len(stages[b]["grps"]))]

    def run(kind, u):
        if u < 0 or u >= len(units):
            return
        b, g = units[u]
        if kind == "z" and g == 0:
            stages[b]["zero"]()
        stages[b][kind](g)
        if kind == "mm" and g == len(stages[b]["grps"]) - 1:
            stages[b]["out"]()

    for u in range(len(units) + 2):
        run("z", u)
        run("erf", u - 1)
        run("mm", u - 2)


def _mk_stages(tc, b, coefp, zxp, exp_, eyp, axp, osbp, zyp, psp,
               lineBF_t, line4_t, zrow_t, cs, out, xw, yw):
    nc = tc.nc
    coef_t, sbp_t = cs
    st = {}
    if True:
        live = [a for a in range(NT) if xw[a][1] > xw[a][0] and yw[a][1] > yw[a][0]]
        grps = _split_groups(live, NGRP, first_small=(b == 0),
                             last_small=(b == BPC - 1))

        def scol(k, a):  # coef column: k in {0:s, 1:bx, 2:amp'}
            o = k * NT + a
            return coef_t[:, o:o + 1]

        ps = psp.tile([P, 2 * D], F32, tag="ps", name=f"ps{b}")
        offys, offxs = {}, {}
        for g, grp in enumerate(grps):
            offys[g] = np.concatenate(
                [[0], np.cumsum([yw[a][1] - yw[a][0] for a in grp])]).astype(int)
            offxs[g] = np.concatenate(
                [[0], np.cumsum([xw[a][1] - xw[a][0] for a in grp])]).astype(int)
        bufs = {}
        mm_state = {"n": 0,
                    "total": sum(len(_mm_parts(yw[a])) for a in live)}

        def stage_zero():
            # zero ps via a K=1 outer product on PE: start=True opens the
            # accumulation region and writes 0 everywhere
            nc.tensor.matmul(ps[:], lhsT=zrow_t[:, 0:P], rhs=zrow_t[:],
                             start=True, stop=False, skip_group_check=True)

        def stage_z(g):
            grp = grps[g]
            offy, offx = offys[g], offxs[g]
            zyb = zyp.tile([P, int(offy[-1])], F32, name="zyb")
            BANK = 512  # f32 cols per PSUM bank; matmul out can't cross one
            for i, a in enumerate(grp):
                w0, w1 = yw[a]
                o0, o1 = int(offy[i]), int(offy[i + 1])
                cuts = [o0] + [c for c in range((o0 // BANK + 1) * BANK, o1, BANK)] + [o1]
                for c0, c1 in zip(cuts[:-1], cuts[1:]):
                    nc.tensor.matmul(
                        zyb[:, c0:c1],
                        lhsT=sbp_t[:, P * a:P * (a + 1)],
                        rhs=line4_t[:, w0 + (c0 - o0):w0 + (c1 - o0)],
                        start=True, stop=True, skip_group_check=True)
            zxb = zxp.tile([P, int(offx[-1])], BF16, name="zxb")
            for i, a in enumerate(grp):
                w0, w1 = xw[a]
                zeng = nc.vector if (i * ZX_DVE) % 16 < ZX_DVE else nc.gpsimd
                zeng.tensor_scalar(
                    out=zxb[:, int(offx[i]):int(offx[i + 1])],
                    in0=lineBF_t[:, w0:w1],
                    scalar1=scol(0, a), scalar2=scol(1, a),
                    op0=OP.mult, op1=OP.add)
            bufs[g] = (zyb, zxb)

        def stage_erf(g):
            zyb, zxb = bufs[g]
            eyb = eyp.tile([P, int(offys[g][-1])], BF16, name="eyb")
            nc.scalar.activation(out=eyb[:], in_=zyb[:],
                                 func=AF.Derivative_Erf)
            exb = exp_.tile([P, int(offxs[g][-1])], BF16, name="exb")
            nc.scalar.activation(out=exb[:], in_=zxb[:],
                                 func=AF.Derivative_Erf)
            bufs[g] = (eyb, exb)

        def stage_mm(g):
            grp = grps[g]
            eyb, exb = bufs[g]
            offy, offx = offys[g], offxs[g]
            for i, a in enumerate(grp):
                wxa = xw[a][1] - xw[a][0]
                ax = axp.tile([P, wxa], BF16, name="ax", tag="ax")
                eng = nc.vector if (i * AX_DVE) % 16 < AX_DVE else nc.gpsimd
                eng.tensor_scalar(
                    out=ax[:], in0=exb[:, int(offx[i]):int(offx[i + 1])],
                    scalar1=scol(2, a), scalar2=None, op0=OP.mult)
                x0, x1 = xw[a]
                y0 = yw[a][0]
                for (p0, p1, co) in _mm_parts(yw[a]):
                    mm_state["n"] += 1
                    nc.tensor.matmul(
                        ps[p0 - (0 if co == 0 else P):
                           p1 - (0 if co == 0 else P),
                           co + x0:co + x1],
                        lhsT=eyb[:, int(offy[i]) + (p0 - y0):
                                 int(offy[i]) + (p1 - y0)],
                        rhs=ax[:],
                        start=False, stop=(mm_state["n"] == mm_state["total"]),
                        skip_group_check=True,
                        tile_position=(0, p0 % P))

        def stage_out():
            osb = osbp.tile([P, 2 * D], F32, name="osb")
            if b == BPC - 1:
                nc.vector.tensor_copy(osb[:, 0:D], ps[:, 0:D])
                nc.scalar.copy(osb[:, D:2 * D], ps[:, D:2 * D])
            else:
                nc.vector.tensor_copy(osb[:], ps[:])
            nc.sync.dma_start(out[b].rearrange("(h p) x -> p h x", h=2),
                              osb[:])

        st["grps"] = grps
        st["zero"] = stage_zero
        st["z"] = stage_z
        st["erf"] = stage_erf
        st["mm"] = stage_mm
        st["out"] = stage_out
    return st


def _split_groups(live, ngrp, first_small=False, last_small=False):
    """Split tiles into groups; optionally carve a small lead/tail group so
    the pipeline fills faster / drains shorter."""
    live = [int(a) for a in live]
    head = live[:6] if first_small and len(live) > 12 else []
    tail = live[len(live) - 6:] if last_small and len(live) > 12 else []
    mid = live[len(head):len(live) - len(tail)]
    n = max(1, ngrp - (1 if head else 0) - (1 if tail else 0),
            -(-len(mid) // 16))  # cap ~16 tiles/group (PSUM bank budget)
    grps = [list(g) for g in np.array_split(np.array(mid), n) if len(g)]
    if head:
        grps = [head] + grps
    if tail:
        grps = grps + [tail]
    return [[int(a) for a in g] for g in grps]


def _mm_parts(ywin):
    """32-partition slabs (PE quadrant alignment); (p0, p1, ps col offset)."""
    w0, w1 = ywin
    parts = []
    p = w0
    while p < w1:
        p1 = min(p + 32, w1)
        parts.append((p, p1, 0 if p < P else D))
        p = p1
    return parts


# ---------------------------------------------------------------------------
# Entry
# ---------------------------------------------------------------------------

def make_in_maps(line_coords, rot_mats, centers, sigmas, amplitudes):
    line_coords = np.ascontiguousarray(np.asarray(line_coords, np.float32))
    rot_mats = np.ascontiguousarray(np.asarray(rot_mats, np.float32))
    centers = np.ascontiguousarray(np.asarray(centers, np.float32))
    sigmas = np.ascontiguousarray(np.asarray(sigmas, np.float32))
    amplitudes = np.ascontiguousarray(np.asarray(amplitudes, np.float32))
    order, xwins, ywins, coef, sbp4, lineBF, line4 = prep(
        line_coords, rot_mats, centers, sigmas, amplitudes)
    in_maps = []
    for c in range(NCORES):
        s = slice(c * BPC, (c + 1) * BPC)
        in_maps.append({
            "lineBF": lineBF,
            "coef": np.ascontiguousarray(
                coef[s].transpose(1, 0, 2).reshape(P, BPC * 3 * NT)),
            "sbp4": np.ascontiguousarray(
                sbp4[s].transpose(1, 0, 2).reshape(4, BPC * N)),
        })
    return (xwins, ywins), in_maps


def kernel(line_coords, rot_mats, centers, sigmas, amplitudes):
    wins, in_maps = make_in_maps(line_coords, rot_mats, centers, sigmas,
                                 amplitudes)
    nc = build(wins)
    res = run_bass_kernel_spmd(nc, in_maps, list(range(NCORES)))
    return np.concatenate([res.results[c]["out"] for c in range(NCORES)],
                          axis=0)


# revision 28
# speedup vs baseline: 1.0954x; 1.0954x over previous
"""AtomGMMProjector Bass kernel for Trainium2 (8 NeuronCores, SPMD).
v3: 2D-bucketed windows, PE-built z, 5-engine balance.

Math (per batch b):
    out[y, x] = sum_n amp[n] * exp(-zy(n,y)^2) * exp(-zx(n,x)^2)
    z*(n, d)  = s[n]*line[d] + b*[n],  s = 1/(sqrt2*sigma), b* = -c*_n*s

Structure (per 128-atom tile, windows precomputed on host):
  * zy on PE: one K=4 bf16 matmul  {s_hi,s_lo,by_hi,by_lo}^T @ {line,line,1,1}
    -> PSUM f32.  The hi/lo split keeps z exact to ~2^-17 while running at
    the 1-cycle/row bf16 rate (f32 matmuls are 4x slower).
  * zx on DVE: tensor_scalar line*s+bx (bf16 4x mode, f32 per-partition
    scalars).
  * erf' (Derivative_Erf = 2/sqrt(pi) exp(-x^2)): ONE batched ACT op per
    axis per batch (PSUM-source for y, SBUF for x) -> bf16.
  * ax = ex*amp' on DVE/Pool (split tuned); amp' = amp*pi/4 cancels the
    (2/sqrt(pi))^2 from the two erf' factors.
  * Accumulating bf16 matmuls into ONE [128, 512] f32 PSUM bank per batch
    (cols 0:256 = y in [0,128), 256:512 = y in [128,256)).  PE writes land
    on 32-partition quadrant boundaries only, so y-windows have 32-aligned
    low edges and each tile's matmul is emitted per 32-slab.
  * PSUM -> SBUF out-copy on DVE (final batch split DVE/ACT for tail
    latency), single HBM DMA per batch.

Atoms are 2D-bucketed per batch on host (adaptive k-d: median split on the
wider axis), so both axes' windows are narrow.  All coefficients
(s, bx, by, amp') are host-precomputed and DMA'd as contiguous
partition-major blocks.  Windows are unioned over each body-slot's 8
possible batches (core c runs batches 4c..4c+3; slot j serves batches
{j, j+4, ...}), so one SPMD program works for all cores.
"""

import numpy as np
from contextlib import ExitStack

import concourse.bass as bass
import concourse.bacc as bacc
import concourse.mybir as mybir
import concourse.tile as tile
from concourse.bass_utils import run_bass_kernel_spmd

F32 = mybir.dt.float32
BF16 = mybir.dt.bfloat16
AF = mybir.ActivationFunctionType
OP = mybir.AluOpType

import os
B, N, D = 32, 4096, 256
NCORES = 8
BPC = B // NCORES
P = 128
NT = N // P
MARGIN_SIG = float(os.environ.get("K_MARGIN", "3.5"))  # window margin, units of per-tile max sigma
QX = 4                       # x window quantization
QYH = 4                      # y high-edge quantization
QYL = 32                     # y low-edge: PE quadrant alignment
PI_4 = 0.7853981633974483

AX_DVE = int(os.environ.get("K_AX_DVE", "8"))    # per 16 tiles, ax ops on DVE
NGRP = int(os.environ.get("K_NGRP", "2"))        # erf groups per axis per batch
ZY_BUFS = int(os.environ.get("K_ZY_BUFS", "2"))
ZX_DVE = int(os.environ.get("K_ZX_DVE", "16"))   # per 16 tiles, zx ops on DVE (rest Pool)
YPEN = float(os.environ.get("K_YPEN", "1.15"))
PS_BUFS = int(os.environ.get("K_PS_BUFS", "2"))


def _bf(a):
    return a.astype(mybir.dt.np(BF16))


# ---------------------------------------------------------------------------
# Host prep
# ---------------------------------------------------------------------------

def _kd_order(cx, cy, levels=5, ypen=YPEN):
    """Adaptive k-d bucketing: at each node, split at the median of the axis
    whose split minimizes the projected window cost (clamped extents +
    margins, unioned across batches).  One split tree shared by all batches
    (the SPMD program is shared), leaf membership per batch."""
    GRID = 128.0  # grid half-width: windows clamp to [-128, 128)

    def ext(c_ax, nodes):
        # unioned, grid-clamped extent of a node along one axis
        lo = np.mean([np.clip(c_ax[b][i], -GRID, GRID).min()
                      for b, i in enumerate(nodes)])
        hi = np.mean([np.clip(c_ax[b][i], -GRID, GRID).max()
                      for b, i in enumerate(nodes)])
        return hi - lo

    def trysplit(c_ax, nodes):
        lo, hi = [], []
        for b, i in enumerate(nodes):
            srt = i[np.argsort(c_ax[b][i], kind="stable")]
            h = len(srt) // 2
            lo.append(srt[:h])
            hi.append(srt[h:])
        return lo, hi

    def split(nodes, depth):
        if depth == 0:
            return [nodes]
        sx = trysplit(cx, nodes)
        sy = trysplit(cy, nodes)
        # projected cost: sum over both children of (ext_x + ext_y), with the
        # y axis weighted for its 32-alignment penalty
        cxx = sum(ext(cx, s) + ypen * ext(cy, s) for s in sx)
        cyy = sum(ext(cx, s) + ypen * ext(cy, s) for s in sy)
        pick = sx if cxx <= cyy else sy
        return split(pick[0], depth - 1) + split(pick[1], depth - 1)

    leaves = split([np.arange(N) for _ in range(B)], levels)
    order = np.empty((B, N), np.int64)
    for b in range(B):
        order[b] = np.concatenate([leaf[b] for leaf in leaves])
    return order


def prep(line_coords, rot_mats, centers, sigmas, amplitudes):
    line = np.asarray(line_coords, np.float64)
    base, step = line[0], line[1] - line[0]
    cx = np.einsum("bj,bnj->bn", rot_mats[:, 0, :], centers).astype(np.float64)
    cy = np.einsum("bj,bnj->bn", rot_mats[:, 1, :], centers).astype(np.float64)
    order = _kd_order(cx, cy)

    sx = np.take_along_axis(cx, order, 1)
    sy = np.take_along_axis(cy, order, 1)
    ss = np.asarray(sigmas, np.float64)[order]
    sa = np.asarray(amplitudes, np.float64)[order]

    # per body-slot windows: slot j is run for batches {j, j+BPC, ...}
    def win(c, slot_batches, q0, q1):
        wins = []
        for t in range(NT):
            seg = slice(P * t, P * (t + 1))
            m = MARGIN_SIG * ss[slot_batches, seg].max(axis=1)
            lo = ((c[slot_batches, seg].min(axis=1) - m - base) / step).min()
            hi = ((c[slot_batches, seg].max(axis=1) + m - base) / step).max()
            w0 = max(0, min(D, int(np.floor(lo / q0) * q0)))
            w1 = max(0, min(D, int(np.ceil((hi + 1.0) / q1) * q1)))
            wins.append((w0, w1))
        return wins

    xwins, ywins = [], []
    for j in range(BPC):
        sb = list(range(j, B, BPC))
        xwins.append(win(sx, sb, QX, QX))
        ywins.append(win(sy, sb, QYL, QYH))

    s = 1.0 / (np.sqrt(2.0) * ss)
    bx = -sx * s
    by = -sy * s
    ap = sa * PI_4

    # coef [B, 128, 3*NT] f32: blocks [s | bx | amp'], atom (t, p) = 128t+p
    coef = np.empty((B, P, 3 * NT), np.float32)
    for k, v in enumerate((s, bx, ap)):
        coef[:, :, k * NT:(k + 1) * NT] = v.reshape(B, NT, P).transpose(0, 2, 1)

    # sbp4 [B, 4, N] bf16: rows {s_hi, s_lo, by_hi, by_lo} (PE zy lhsT)
    s_hi = _bf(s).astype(np.float64)
    by_hi = _bf(by).astype(np.float64)
    sbp4 = np.stack([s_hi, s - s_hi, by_hi, by - by_hi], axis=1)
    sbp4 = _bf(sbp4.astype(np.float32))                     # (B, 4, N)
    # line4 rows {line, line, 1, 1} share sbp4's partition count: packed as
    # its leading D columns so ONE DMA gates everything the zy matmuls need

    lineBF = _bf(np.tile(np.asarray(line_coords, np.float32)[None, :], (P, 1)))
    l32 = np.asarray(line_coords, np.float32)
    line4 = _bf(np.stack([l32, l32, np.ones(D, np.float32),
                          np.ones(D, np.float32)], axis=0))  # (4, D)
    return order, xwins, ywins, coef, sbp4, lineBF, line4


# ---------------------------------------------------------------------------
# Device program
# ---------------------------------------------------------------------------

def build(wins, repeats=1):
    xwins, ywins = wins
    nc = bacc.Bacc("TRN2", target_bir_lowering=False, debug=False)
    coef_d = nc.dram_tensor("coef", [P, D // 2 + BPC * 3 * NT], F32, kind="ExternalInput")
    sbp4_d = nc.dram_tensor("sbp4", [4, D + BPC * N], BF16, kind="ExternalInput")
    out_d = nc.dram_tensor("out", [BPC, D, D], F32, kind="ExternalOutput")

    with tile.TileContext(nc) as tc, ExitStack() as ctx:
        nc_ = tc.nc
        constp = ctx.enter_context(tc.tile_pool(name="const", bufs=1))
        coefp = ctx.enter_context(tc.tile_pool(name="coef", bufs=BPC))
        zxp = ctx.enter_context(tc.tile_pool(name="zx", bufs=3))
        exp_ = ctx.enter_context(tc.tile_pool(name="ex", bufs=3))
        eyp = ctx.enter_context(tc.tile_pool(name="ey", bufs=3))
        axp = ctx.enter_context(tc.tile_pool(name="ax", bufs=12))
        osbp = ctx.enter_context(tc.tile_pool(name="osb", bufs=2))
        zyp = ctx.enter_context(tc.tile_pool(name="zy", bufs=ZY_BUFS, space="PSUM"))
        psp = ctx.enter_context(tc.tile_pool(name="ps", bufs=PS_BUFS, space="PSUM"))

        zrow_t = constp.tile([1, 2 * D], BF16)

        for _ in range(repeats):
            _body(tc, coefp, zxp, exp_, eyp, axp, osbp, zyp, psp,
                  zrow_t, coef_d.ap(), sbp4_d.ap(),
                  out_d.ap(), xwins, ywins)
    nc.compile()
    return nc


def _body(tc, coefp, zxp, exp_, eyp, axp, osbp, zyp, psp,
          zrow_t, coef, sbp4, out, xwins, ywins):
    nc = tc.nc

    # coefficient DMAs, split so batch 0's first tiles land first: the
    # first zy/zx work can start while the bulk transfer is still in flight
    sbp_all = coefp.tile([4, D + BPC * N], BF16, tag="sbp")
    CUT = D + 2 * N
    nc.sync.dma_start(sbp_all[:, 0:CUT], sbp4[:, 0:CUT])
    nc.vector.memset(zrow_t[:], 0.0)
    line4_t = sbp_all
    HC = D // 2  # line row, bf16 bit-packed into f32 columns
    coef_all = coefp.tile([P, HC + BPC * 3 * NT], F32, tag="coef")
    nc.sync.dma_start(coef_all[:, 0:HC + 6 * NT], coef[:, 0:HC + 6 * NT])
    nc.sync.dma_start(sbp_all[:, CUT:], sbp4[:, CUT:])
    nc.sync.dma_start(coef_all[:, HC + 6 * NT:], coef[:, HC + 6 * NT:])
    lineBF_t = coef_all[:, 0:HC].bitcast(BF16)
    
    per_batch = [(coef_all[:, HC + b * 3 * NT:HC + (b + 1) * 3 * NT],
                  sbp_all[:, D + b * N:D + (b + 1) * N])
                 for b in range(BPC)]

    stages = [_mk_stages(tc, b, coefp, zxp, exp_, eyp, axp, osbp, zyp, psp,
                         lineBF_t, line4_t, zrow_t, per_batch[b], out,
                         xwins[b], ywins[b]) for b in range(BPC)]
    # ps zeroing for the first two batches up-front: PE-only, no DMA deps,
    # runs during the coefficient transfers and warms the PE p-state
    stages[0]["zero"]()
    # software pipeline over (batch, group) units, depth 2: each engine's
    # in-order stream has unit u's producers (PE zy / DVE zx) ahead of unit
    # u-1's erf (ACT) ahead of unit u-2's consumers (ax+slab matmuls), so
    # ACT never waits on a producer and PE never runs dry
    units = [(b, g) for b in range(BPC)
             for g in range(len(stages[b]["grps"]))]

    def run(kind, u):
        if u < 0 or u >= len(units):
            return
        b, g = units[u]
        if kind == "z" and g == 0 and b >= 1:
            stages[b]["zero"]()
        stages[b][kind](g)
        if kind == "mm" and g == len(stages[b]["grps"]) - 1:
            stages[b]["out"]()

    for u in range(len(units) + 2):
        run("z", u)
        run("erf", u - 1)
        run("mm", u - 2)


def _mk_stages(tc, b, coefp, zxp, exp_, eyp, axp, osbp, zyp, psp,
               lineBF_t, line4_t, zrow_t, cs, out, xw, yw):
    nc = tc.nc
    coef_t, sbp_t = cs
    st = {}
    if True:
        live = [a for a in range(NT) if xw[a][1] > xw[a][0] and yw[a][1] > yw[a][0]]
        if b == BPC - 1:
            # pure-half-1 tiles (y-window entirely >= 128) last, so half 0 of
            # the output can flush while their matmuls still run
            live.sort(key=lambda a: yw[a][0] >= P)
        grps = _split_groups(live, NGRP, first_small=(b == 0),
                             last_small=(b == BPC - 1))

        def scol(k, a):  # coef column: k in {0:s, 1:bx, 2:amp'}
            o = k * NT + a
            return coef_t[:, o:o + 1]

        ps = psp.tile([P, 2 * D], F32, tag="ps", name=f"ps{b}")
        offys, offxs = {}, {}
        for g, grp in enumerate(grps):
            offys[g] = np.concatenate(
                [[0], np.cumsum([yw[a][1] - yw[a][0] for a in grp])]).astype(int)
            offxs[g] = np.concatenate(
                [[0], np.cumsum([xw[a][1] - xw[a][0] for a in grp])]).astype(int)
        bufs = {}
        mm_state = {"n": 0,
                    "total": sum(len(_mm_parts(yw[a])) for a in live)}
        lastg0 = max((g for g, grp in enumerate(grps)
                      if any(yw[a][0] < P for a in grp)), default=-1)
        split_out = (b == BPC - 1) and lastg0 < len(grps) - 1

        def stage_zero():
            # zero ps via a K=1 outer product on PE: start=True opens the
            # accumulation region and writes 0 everywhere
            nc.tensor.matmul(ps[:], lhsT=zrow_t[:, 0:P], rhs=zrow_t[:],
                             start=True, stop=False, skip_group_check=True)

        def stage_z(g):
            grp = grps[g]
            offy, offx = offys[g], offxs[g]
            zyb = zyp.tile([P, int(offy[-1])], F32, name="zyb")
            BANK = 512  # f32 cols per PSUM bank; matmul out can't cross one
            for i, a in enumerate(grp):
                w0, w1 = yw[a]
                o0, o1 = int(offy[i]), int(offy[i + 1])
                cuts = [o0] + [c for c in range((o0 // BANK + 1) * BANK, o1, BANK)] + [o1]
                for c0, c1 in zip(cuts[:-1], cuts[1:]):
                    nc.tensor.matmul(
                        zyb[:, c0:c1],
                        lhsT=sbp_t[:, P * a:P * (a + 1)],
                        rhs=line4_t[:, w0 + (c0 - o0):w0 + (c1 - o0)],
                        start=True, stop=True, skip_group_check=True)
            zxb = zxp.tile([P, int(offx[-1])], BF16, name="zxb")
            for i, a in enumerate(grp):
                w0, w1 = xw[a]
                zeng = nc.vector if (i * ZX_DVE) % 16 < ZX_DVE else nc.gpsimd
                zeng.tensor_scalar(
                    out=zxb[:, int(offx[i]):int(offx[i + 1])],
                    in0=lineBF_t[:, w0:w1],
                    scalar1=scol(0, a), scalar2=scol(1, a),
                    op0=OP.mult, op1=OP.add)
            bufs[g] = (zyb, zxb)

        def stage_erf(g):
            zyb, zxb = bufs[g]
            eyb = eyp.tile([P, int(offys[g][-1])], BF16, name="eyb")
            nc.scalar.activation(out=eyb[:], in_=zyb[:],
                                 func=AF.Derivative_Erf)
            exb = exp_.tile([P, int(offxs[g][-1])], BF16, name="exb")
            nc.scalar.activation(out=exb[:], in_=zxb[:],
                                 func=AF.Derivative_Erf)
            bufs[g] = (eyb, exb)

        def stage_mm(g):
            grp = grps[g]
            eyb, exb = bufs[g]
            offy, offx = offys[g], offxs[g]
            for i, a in enumerate(grp):
                wxa = xw[a][1] - xw[a][0]
                ax = axp.tile([P, wxa], BF16, name="ax", tag="ax")
                eng = nc.vector if (i * AX_DVE) % 16 < AX_DVE else nc.gpsimd
                eng.tensor_scalar(
                    out=ax[:], in0=exb[:, int(offx[i]):int(offx[i + 1])],
                    scalar1=scol(2, a), scalar2=None, op0=OP.mult)
                x0, x1 = xw[a]
                y0 = yw[a][0]
                for (p0, p1, co) in _mm_parts(yw[a]):
                    mm_state["n"] += 1
                    nc.tensor.matmul(
                        ps[p0 - (0 if co == 0 else P):
                           p1 - (0 if co == 0 else P),
                           co + x0:co + x1],
                        lhsT=eyb[:, int(offy[i]) + (p0 - y0):
                                 int(offy[i]) + (p1 - y0)],
                        rhs=ax[:],
                        start=False, stop=(mm_state["n"] == mm_state["total"]),
                        skip_group_check=True,
                        tile_position=(0, p0 % P))
            if split_out and g == lastg0:
                # all later groups only touch y>=128: flush half 0 now so its
                # copy+DMA overlap the remaining matmuls
                osb0 = osbp.tile([P, D], F32, name="osb0", tag="osb0")
                nc.vector.tensor_copy(osb0[:], ps[:, 0:D])
                ov = out[b].rearrange("(h p) x -> p h x", h=2)
                nc.sync.dma_start(ov[:, 0:1],
                                  osb0[:].rearrange("p (h x) -> p h x", h=1))

        def stage_out():
            ov = out[b].rearrange("(h p) x -> p h x", h=2)
            if split_out:
                # half 0 already flushed after its last matmul group
                osb = osbp.tile([P, D], F32, name="osb")
                nc.scalar.copy(osb[:], ps[:, D:2 * D])
                nc.sync.dma_start(ov[:, 1:2],
                                  osb[:].rearrange("p (h x) -> p h x", h=1))
            elif b == BPC - 1:
                osb = osbp.tile([P, 2 * D], F32, name="osb")
                nc.vector.tensor_copy(osb[:, 0:D], ps[:, 0:D])
                nc.sync.dma_start(ov[:, 0:1], osb[:, 0:D].rearrange("p (h x) -> p h x", h=1))
                nc.scalar.copy(osb[:, D:2 * D], ps[:, D:2 * D])
                nc.sync.dma_start(ov[:, 1:2], osb[:, D:2 * D].rearrange("p (h x) -> p h x", h=1))
            else:
                osb = osbp.tile([P, 2 * D], F32, name="osb")
                nc.vector.tensor_copy(osb[:], ps[:])
                nc.sync.dma_start(ov, osb[:])

        st["grps"] = grps
        st["zero"] = stage_zero
        st["z"] = stage_z
        st["erf"] = stage_erf
        st["mm"] = stage_mm
        st["out"] = stage_out
    return st


def _split_groups(live, ngrp, first_small=False, last_small=False):
    """Split tiles into groups; optionally carve a small lead/tail group so
    the pipeline fills faster / drains shorter."""
    live = [int(a) for a in live]
    HEADN = int(os.environ.get("K_HEAD", "8"))
    head = live[:HEADN] if first_small and len(live) > 12 else []
    TAILN = int(os.environ.get("K_TAIL", "10"))
    tail = live[len(live) - TAILN:] if last_small and len(live) > 12 else []
    mid = live[len(head):len(live) - len(tail)]
    n = max(1, ngrp - (1 if head else 0) - (1 if tail else 0),
            -(-len(mid) // 16))  # cap ~16 tiles/group (PSUM bank budget)
    grps = [list(g) for g in np.array_split(np.array(mid), n) if len(g)]
    if head:
        grps = [head] + grps
    if tail:
        grps = grps + [tail]
    return [[int(a) for a in g] for g in grps]


def _mm_parts(ywin):
    """32-partition slabs (PE quadrant alignment); (p0, p1, ps col offset)."""
    w0, w1 = ywin
    parts = []
    p = w0
    while p < w1:
        p1 = min(p + 32, w1)
        parts.append((p, p1, 0 if p < P else D))
        p = p1
    return parts


# ---------------------------------------------------------------------------
# Entry
# ---------------------------------------------------------------------------

def make_in_maps(line_coords, rot_mats, centers, sigmas, amplitudes):
    line_coords = np.ascontiguousarray(np.asarray(line_coords, np.float32))
    rot_mats = np.ascontiguousarray(np.asarray(rot_mats, np.float32))
    centers = np.ascontiguousarray(np.asarray(centers, np.float32))
    sigmas = np.ascontiguousarray(np.asarray(sigmas, np.float32))
    amplitudes = np.ascontiguousarray(np.asarray(amplitudes, np.float32))
    order, xwins, ywins, coef, sbp4, lineBF, line4 = prep(
        line_coords, rot_mats, centers, sigmas, amplitudes)
    in_maps = []
    for c in range(NCORES):
        s = slice(c * BPC, (c + 1) * BPC)
        in_maps.append({
            "coef": np.ascontiguousarray(np.concatenate(
                [lineBF.view(np.uint16).reshape(P, D // 2, 2)
                 .view(np.uint32).reshape(P, D // 2).view(np.float32),
                 coef[s].transpose(1, 0, 2).reshape(P, BPC * 3 * NT)],
                axis=1)),
            "sbp4": np.ascontiguousarray(np.concatenate(
                [line4, sbp4[s].transpose(1, 0, 2).reshape(4, BPC * N)],
                axis=1)),
        })
    return (xwins, ywins), in_maps


def kernel(line_coords, rot_mats, centers, sigmas, amplitudes):
    wins, in_maps = make_in_maps(line_coords, rot_mats, centers, sigmas,
                                 amplitudes)
    nc = build(wins)
    res = run_bass_kernel_spmd(nc, in_maps, list(range(NCORES)))
    return np.concatenate([res.results[c]["out"] for c in range(NCORES)],
                          axis=0)
